# revision 56
# baseline (speedup 1.0000x reference)
"""Trainium2 Bass kernel for nn_MultiHeadAttention_KT (causal linear attention).

Math (per batch b):
  q' = leaky((q*qm) @ Wq + bq); k' = leaky((k*km) @ Wk + bk); v' = (v*vm) @ Wv
  per head h (DEPTH=64):   S_t = sum_{s<=t} k_s v_s^T ; z_t = sum_{s<=t} k_s
                           attn_t = (q_t @ S_t) / (q_t . z_t)
  out = concat_heads(attn) @ Wo + bo

Sharding: 8 cores = 2 batches x 4 head-groups (4 heads / 256 cols each).
Host transposes inputs (xq = (q*qm)^T etc.), casts everything to bf16,
and sums the 4 partial output projections per batch.

All matmuls run in bf16 (1 cycle/row on PE; f32 accumulate in PSUM).
Transposes (K -> natural layout, attn -> attn^T) run on the DMA xbar in
batched [128, 256..512] -> 3D form, not on the PE.

The three phases are emitted software-pipelined (Tile's schedule is
static, so program order decides overlap):
  proj s-chunk sc -> attention chunks 4sc..4sc+3 -> output block sc-1
so the sequential attention state chain hides under projection matmuls
and the output projection streams out behind attention.

Chunked linear attention on device (chunk C=128, all matmuls on PE):
  AT   = K Q^T (per chunk, [s,t] layout)      masked with triu (s<=t)
  num  = ATm^T V_aug + Q S_aug                (V_aug = [V | 1], S_aug = [S | z])
  attn = num[:, :64] * (1/num[:, 64])
  S_aug += K_chunk^T V_aug    (delta matmul; f32 master state + bf16 copy)
"""

import os
import sys

sys.path.insert(0, "/opt/trn_rl_repo")

import numpy as np
import ml_dtypes

BF16 = ml_dtypes.bfloat16

B, S, D, H = 2, 2048, 1024, 16
DEPTH = 64
N_CORES = 8
HPC = 4                 # heads per core
JS = HPC * DEPTH        # 256 projected columns per core
C = 128                 # attention chunk length
NCH = S // C            # 16 chunks
IB = D // 128           # 8 contraction blocks
SCH = 512               # projection s-chunk == output s-chunk == load chunk
NSC = S // SCH          # 4 projection chunks
JAUG = DEPTH + 1        # 65 (V augmented with ones column)

TRACE = False           # set True from test harness to capture NTFF profile
TRACE_CORES = None
LAST_RESULTS = None     # BassKernelResults of the last kernel() call

_PROG = None


def _build():
    import concourse.bacc as bacc
    import concourse.mybir as mybir
    import concourse.tile as tile

    dt = mybir.dt
    f32 = dt.float32
    bf16 = dt.bfloat16
    AF = mybir.ActivationFunctionType
    Alu = mybir.AluOpType

    nc = bacc.Bacc("TRN2", target_bir_lowering=False, debug=False,
                   num_devices=N_CORES)

    xq = nc.dram_tensor("xq", [D, S], bf16, kind="ExternalInput").ap()
    xk = nc.dram_tensor("xk", [D, S], bf16, kind="ExternalInput").ap()
    xv = nc.dram_tensor("xv", [D, S], bf16, kind="ExternalInput").ap()
    wq = nc.dram_tensor("wq", [D, JS], bf16, kind="ExternalInput").ap()
    wk = nc.dram_tensor("wk", [D, JS], bf16, kind="ExternalInput").ap()
    wv = nc.dram_tensor("wv", [D, JS], bf16, kind="ExternalInput").ap()
    wo = nc.dram_tensor("wo", [JS, D], bf16, kind="ExternalInput").ap()
    bqd = nc.dram_tensor("bq", [2, 128], f32, kind="ExternalInput").ap()
    bkd = nc.dram_tensor("bk", [2, 128], f32, kind="ExternalInput").ap()
    triu = nc.dram_tensor("triu", [128, 256], f32, kind="ExternalInput").ap()
    ident = nc.dram_tensor("ident", [128, 128], bf16, kind="ExternalInput").ap()
    po = nc.dram_tensor("po", [D, S], bf16, kind="ExternalOutput").ap()

    def mm(out, lhsT, rhs, **kw):
        nc.tensor.matmul(out, lhsT, rhs, **kw)

    with tile.TileContext(nc) as tc:
        with (
            tc.tile_pool(name="persist", bufs=1) as pp,
            tc.tile_pool(name="xin", bufs=NSC) as xpool,
            tc.tile_pool(name="work", bufs=4) as wk_pool,
            tc.tile_pool(name="outp", bufs=3) as opool,
            tc.tile_pool(name="psA", bufs=2, space="PSUM") as psA,
            tc.tile_pool(name="psAT", bufs=2, space="PSUM") as psAT,
            tc.tile_pool(name="psB", bufs=2, space="PSUM") as psB,
            tc.tile_pool(name="psS", bufs=1, space="PSUM") as psS,
        ):
            # ---- tiles ------------------------------------------------------
            wq_sb = pp.tile([128, IB, JS], bf16, tag="wq", name="wq_sb")
            wk_sb = pp.tile([128, IB, JS], bf16, tag="wk", name="wk_sb")
            wv_sb = pp.tile([128, IB, JS], bf16, tag="wv", name="wv_sb")
            wo_sb = pp.tile([128, 2, D], bf16, tag="wo", name="wo_sb")
            bq_sb = pp.tile([128, 2], f32, tag="bq", name="bq_sb")
            bk_sb = pp.tile([128, 2], f32, tag="bk", name="bk_sb")
            triu_sb = pp.tile([128, 256], f32, tag="triu", name="triu_sb")
            ident_sb = pp.tile([128, 128], bf16, tag="ident", name="ident_sb")

            qT_sb = [pp.tile([128, S], bf16, tag=f"qT{jb}", name=f"qT{jb}") for jb in range(2)]
            kT_sb = [pp.tile([128, S], bf16, tag=f"kT{jb}", name=f"kT{jb}") for jb in range(2)]
            # attn^T, [128 j, jb, s]
            aT_sb = pp.tile([128, 2, S], bf16, tag="aT", name="aT_sb")
            # K natural per jb: [128 s%128, chunk, 128 j] (2 heads along j)
            knat_sb = [pp.tile([128, NCH, 128], bf16, tag=f"knat{jb}",
                               name=f"knat{jb}")
                       for jb in range(2)]
            vaug_sb = [pp.tile([128, HPC * JAUG], bf16, tag=f"vaug{i}",
                               name=f"vaug{i}")
                       for i in range(NCH)]
            # state accumulates directly in PSUM (one bank per jb, two heads
            # per bank at partitions (h%2)*64..+64); bf16 shadow in SBUF
            s_ps = [psS.tile([128, JAUG], f32, tag=f"sps{jb}", name=f"sps{jb}")
                    for jb in range(2)]
            saug_bf = [pp.tile([128, JAUG], bf16, tag=f"saugb{jb}",
                               name=f"saugb{jb}")
                       for jb in range(2)]

            xq_ts = [xpool.tile([128, IB, SCH], bf16, tag="xq", name=f"xq_t{i}")
                     for i in range(NSC)]
            xk_ts = [xpool.tile([128, IB, SCH], bf16, tag="xk", name=f"xk_t{i}")
                     for i in range(NSC)]
            xv_ts = [xpool.tile([128, IB, SCH], bf16, tag="xv", name=f"xv_t{i}")
                     for i in range(NSC)]

            xq_r = xq.rearrange("(ib p) s -> p ib s", p=128)
            xk_r = xk.rearrange("(ib p) s -> p ib s", p=128)
            xv_r = xv.rearrange("(ib p) s -> p ib s", p=128)

            # ---- loads, criticality-ordered --------------------------------
            # First-chunk loads split in half (by contraction block) so each
            # lands on more parallel DMA queues and the first matmul group
            # can start sooner.
            HB = IB // 2
            nc.sync.dma_start(ident_sb[:], ident)
            nc.sync.dma_start(xq_ts[0][:, 0:HB], xq_r[:, 0:HB, 0:SCH])
            nc.sync.dma_start(xq_ts[0][:, HB:IB], xq_r[:, HB:IB, 0:SCH])
            nc.sync.dma_start(wq_sb[:], wq.rearrange("(ib p) j -> p ib j", p=128))
            nc.sync.dma_start(bq_sb[:], bqd.rearrange("jb p -> p jb"))
            nc.sync.dma_start(xk_ts[0][:, 0:HB], xk_r[:, 0:HB, 0:SCH])
            nc.sync.dma_start(xk_ts[0][:, HB:IB], xk_r[:, HB:IB, 0:SCH])
            nc.sync.dma_start(wk_sb[:], wk.rearrange("(ib p) j -> p ib j", p=128))
            nc.sync.dma_start(bk_sb[:], bkd.rearrange("jb p -> p jb"))
            nc.sync.dma_start(xv_ts[0][:, 0:HB], xv_r[:, 0:HB, 0:SCH])
            nc.sync.dma_start(xv_ts[0][:, HB:IB], xv_r[:, HB:IB, 0:SCH])
            nc.sync.dma_start(wv_sb[:], wv.rearrange("(ib p) j -> p ib j", p=128))
            nc.sync.dma_start(triu_sb[:], triu)
            for sc in range(1, NSC):
                s0 = sc * SCH
                nc.sync.dma_start(xq_ts[sc][:], xq_r[:, :, s0:s0 + SCH])
                nc.sync.dma_start(xk_ts[sc][:], xk_r[:, :, s0:s0 + SCH])
                nc.sync.dma_start(xv_ts[sc][:], xv_r[:, :, s0:s0 + SCH])
            nc.sync.dma_start(wo_sb[:], wo.rearrange("(jb p) o -> p jb o", p=128))

            # ---- phase bodies ----------------------------------------------
            def qk_group(sc, name, jb):
                s0 = sc * SCH
                w_sb, x_t, b_sb, dst = (
                    (wq_sb, xq_ts[sc], bq_sb, qT_sb) if name == "q"
                    else (wk_sb, xk_ts[sc], bk_sb, kT_sb))
                ps = psA.tile([128, SCH], f32, tag="A")
                for ib in range(IB):
                    mm(ps[:], w_sb[:, ib, jb * 128:(jb + 1) * 128],
                       x_t[:, ib, :],
                       start=(ib == 0), stop=(ib == IB - 1))
                nc.scalar.activation(
                    dst[jb][:, s0:s0 + SCH], ps[:], AF.Prelu,
                    bias=b_sb[:, jb:jb + 1], scale=1.0, alpha=0.1)

            def v_group(sc, ss):
                # v' natural: psum [128 s, JS], augmented store
                ps = psA.tile([128, JS], f32, tag="A")
                for ib in range(IB):
                    mm(ps[:], xv_ts[sc][:, ib, ss * 128:(ss + 1) * 128],
                       wv_sb[:, ib, :],
                       start=(ib == 0), stop=(ib == IB - 1))
                vt = vaug_sb[sc * (SCH // 128) + ss]
                vt_r = vt[:].rearrange("p (h e) -> p h e", h=HPC)
                nc.scalar.activation(
                    vt_r[:, :, 0:DEPTH],
                    ps[:].rearrange("p (h e) -> p h e", h=HPC), AF.Copy)
                nc.gpsimd.memset(vt_r[:, :, DEPTH:JAUG], 1.0)

            def attn_chunk(ci):
                scol = ci * C
                attn2 = wk_pool.tile([128, 2, 2 * DEPTH], bf16, tag="attn2",
                                     name=f"attn2_{ci}")
                for jb in range(2):
                    # K natural for this chunk (both heads at once) via PE
                    # transpose — the DMA xbar would serialize against the
                    # in-flight x loads on the sync queues.
                    if ci < NCH - 1:
                        knT_ps = psAT.tile([128, 128], bf16, tag="AT")
                        nc.tensor.transpose(
                            knT_ps[:], kT_sb[jb][:, scol:scol + C],
                            ident_sb[:])
                        if jb == 0:
                            nc.vector.tensor_copy(knat_sb[jb][:, ci, :],
                                                  knT_ps[:])
                        else:
                            nc.scalar.activation(knat_sb[jb][:, ci, :],
                                                 knT_ps[:], AF.Copy)
                    for hh in range(2):
                        h = jb * 2 + hh
                        jo = hh * DEPTH
                        kT_v = kT_sb[jb][jo:jo + DEPTH, scol:scol + C]
                        qT_v = qT_sb[jb][jo:jo + DEPTH, scol:scol + C]
                        vt = vaug_sb[ci][:, h * JAUG:(h + 1) * JAUG]

                        # AT = K Q^T  [s, t]; mask s<=t
                        at_ps = psAT.tile([128, C], f32, tag="AT")
                        mm(at_ps[:], kT_v, qT_v, start=True, stop=True)
                        atm = wk_pool.tile([128, C], bf16, tag="atm",
                                           name=f"atm_{ci}_{h}")
                        nc.vector.tensor_tensor(atm[:], at_ps[:],
                                                triu_sb[:, 0:C],
                                                op=Alu.mult)

                        # num[t, 0:64] + den[t, 64]
                        num_ps = psB.tile([128, JAUG], f32, tag="B")
                        mm(num_ps[:], atm[:], vt, start=True, stop=(ci == 0))
                        if ci > 0:
                            mm(num_ps[:], qT_v,
                               saug_bf[jb][jo:jo + DEPTH, :],
                               start=False, stop=True)

                        recip = wk_pool.tile([128, 1], f32, tag="recip",
                                             name=f"recip_{ci}_{h}")
                        nc.vector.reciprocal(recip[:], num_ps[:, DEPTH:JAUG])
                        nc.scalar.activation(attn2[:, jb, jo:jo + DEPTH],
                                             num_ps[:, 0:DEPTH], AF.Copy,
                                             scale=recip[:])

                        # state delta accumulates straight into the
                        # persistent PSUM state bank
                        if ci < NCH - 1:
                            mm(s_ps[jb][jo:jo + DEPTH, :],
                               knat_sb[jb][:, ci, jo:jo + DEPTH], vt,
                               start=(ci == 0), stop=True,
                               skip_group_check=True)

                    # refresh the bf16 shadow for the next chunk's num matmul
                    if ci < NCH - 1:
                        nc.vector.tensor_copy(saug_bf[jb][:], s_ps[jb][:])

                if ci < 4 * (NSC - 2):
                    # attn^T for all 4 heads in one batched xbar transpose
                    nc.sync.dma_start_transpose(
                        aT_sb[:, :, scol:scol + C],
                        attn2[:].rearrange("p a b -> p (a b)"))
                else:
                    # tail chunks: transpose on the PE instead — the last
                    # output block would otherwise stall ~6us per chunk on
                    # xbar-transpose completion (mode-switch serialization
                    # against the tail store DMAs)
                    for jb in range(2):
                        at2_ps = psAT.tile([128, 128], bf16, tag="AT")
                        nc.tensor.transpose(at2_ps[:], attn2[:, jb, :],
                                            ident_sb[:])
                        if jb == 0:
                            nc.vector.tensor_copy(
                                aT_sb[:, 0, scol:scol + C], at2_ps[:])
                        else:
                            nc.scalar.activation(
                                aT_sb[:, 1, scol:scol + C], at2_ps[:],
                                AF.Copy)

            po_r = po.rearrange("(g ob p) s -> g p ob s", ob=4, p=128)

            def po_compute(sq, g):
                c0 = sq * SCH
                ot = opool.tile([128, 4, SCH], bf16, tag="ot",
                                name=f"ot_{sq}_{g}")
                for oo in range(4):
                    ob = g * 4 + oo
                    ps = psA.tile([128, SCH], f32, tag="A")
                    for jb in range(2):
                        mm(ps[:], wo_sb[:, jb, ob * 128:(ob + 1) * 128],
                           aT_sb[:, jb, c0:c0 + SCH],
                           start=(jb == 0), stop=(jb == 1))
                    if oo % 2 == 0:
                        nc.scalar.activation(ot[:, oo, :], ps[:], AF.Copy)
                    else:
                        nc.vector.tensor_copy(ot[:, oo, :], ps[:])
                return ot

            def po_store(sq, g, ot, store_engine=None):
                c0 = sq * SCH
                (store_engine or nc.gpsimd).dma_start(
                    po_r[g, :, :, c0:c0 + SCH], ot[:])

            def po_half(sq, g, store_engine=None):
                po_store(sq, g, po_compute(sq, g), store_engine)

            # ---- software-pipelined emission -------------------------------
            # PE warmup: dummy matmuls on the identity while the x loads
            # stream in, so the HAM clock gate opens (1.2 -> 2.4 GHz)
            # before the first real matmul group.
            wu_ps = psAT.tile([128, 128], f32, tag="AT", name="warmup_ps")
            NWU = 40
            for i in range(NWU):
                mm(wu_ps[:], ident_sb[:], ident_sb[:],
                   start=(i == 0), stop=(i == NWU - 1))

            # Lag-1 fine weave: during proj s-chunk sc, interleave the four
            # attention chunks of group sc-1 between projection psum-groups
            # (each ~1.7us of independent PE work hides the cross-engine
            # links of the attention state chain in the static schedule),
            # plus the output-projection halves of group sc-2.
            for sc in range(NSC):
                a = 4 * (sc - 1)
                qk_group(sc, "q", 0)
                if sc >= 1: attn_chunk(a + 0)
                qk_group(sc, "q", 1)
                if sc >= 1: attn_chunk(a + 1)
                qk_group(sc, "k", 0)
                if sc >= 1: attn_chunk(a + 2)
                qk_group(sc, "k", 1)
                if sc >= 1: attn_chunk(a + 3)
                v_group(sc, 0)
                v_group(sc, 1)
                if sc >= 2: po_half(sc - 2, 0)
                v_group(sc, 2)
                v_group(sc, 3)
                if sc >= 2: po_half(sc - 2, 1)
            # drain the pipeline: the last attention group fully FIRST, then
            # the remaining output blocks with their stores on the SYNC
            # engine. Sync executes its stream in order, so the stores queue
            # strictly after the tail xbar transposes (one mode switch) —
            # on gpsimd the scheduler slides them between the transposes and
            # every transpose<->copy transition serializes the DMA fabric
            # (~6us each, measured).
            # po(NSC-2) compute interleaves between the tail attention
            # chunks (its aT inputs are ready) to fill the chain stalls;
            # ALL tail stores defer past the last xbar transpose so the
            # sync queue sees exactly one transpose->copy mode switch.
            attn_chunk(4 * NSC - 4)
            ot20 = po_compute(NSC - 2, 0)
            attn_chunk(4 * NSC - 3)
            ot21 = po_compute(NSC - 2, 1)
            attn_chunk(4 * NSC - 2)
            attn_chunk(4 * NSC - 1)
            po_store(NSC - 2, 0, ot20, store_engine=nc.sync)
            po_store(NSC - 2, 1, ot21, store_engine=nc.sync)
            po_half(NSC - 1, 0, store_engine=nc.sync)
            # final block: store in two halves so the last DMA is smaller
            # and starts as soon as its two copies are done
            otf = po_compute(NSC - 1, 1)
            c0 = (NSC - 1) * SCH
            nc.sync.dma_start(po_r[1, :, 0:2, c0:c0 + SCH], otf[:, 0:2, :])
            nc.sync.dma_start(po_r[1, :, 2:4, c0:c0 + SCH], otf[:, 2:4, :])

    nc.compile()
    return nc


def _get_prog():
    global _PROG
    if _PROG is None:
        _PROG = _build()
    return _PROG


def kernel(q, k, v, query_mask, key_mask, value_mask,
           Wq, bq, Wk, bk, Wv, bv, Wo, bo):
    global LAST_RESULTS
    from concourse import bass_utils

    q = np.asarray(q, np.float32)
    k = np.asarray(k, np.float32)
    v = np.asarray(v, np.float32)
    qm = q * np.asarray(query_mask, np.float32)
    km = k * np.asarray(key_mask, np.float32)
    vm = v * np.asarray(value_mask, np.float32)
    Wq = np.asarray(Wq, np.float32)
    Wk = np.asarray(Wk, np.float32)
    Wv = np.asarray(Wv, np.float32)
    Wo = np.asarray(Wo, np.float32)
    bq = np.asarray(bq, np.float32)
    bk = np.asarray(bk, np.float32)
    bv = np.asarray(bv, np.float32)
    bo = np.asarray(bo, np.float32)
    assert not np.any(bv), "kernel assumes bv == 0 (true for this problem)"

    nc = _get_prog()

    triu = np.tile(np.triu(np.ones((128, 128), np.float32)), (1, 2))
    ident = np.eye(128, dtype=np.float32).astype(BF16)
    xqs = [np.ascontiguousarray(qm[b].T).astype(BF16) for b in range(B)]
    xks = [np.ascontiguousarray(km[b].T).astype(BF16) for b in range(B)]
    xvs = [np.ascontiguousarray(vm[b].T).astype(BF16) for b in range(B)]

    in_maps = []
    for c in range(N_CORES):
        b, g = divmod(c, HPC)
        js = slice(g * JS, (g + 1) * JS)
        in_maps.append({
            "xq": xqs[b], "xk": xks[b], "xv": xvs[b],
            "wq": np.ascontiguousarray(Wq[:, js]).astype(BF16),
            "wk": np.ascontiguousarray(Wk[:, js]).astype(BF16),
            "wv": np.ascontiguousarray(Wv[:, js]).astype(BF16),
            "wo": np.ascontiguousarray(Wo[js, :]).astype(BF16),
            "bq": np.ascontiguousarray(bq[js].reshape(2, 128)),
            "bk": np.ascontiguousarray(bk[js].reshape(2, 128)),
            "triu": triu, "ident": ident,
        })

    res = bass_utils.run_bass_kernel_spmd(
        nc, in_maps, core_ids=list(range(N_CORES)),
        trace=TRACE, trace_cores=TRACE_CORES)
    LAST_RESULTS = res

    out = np.zeros((B, S, D), np.float32)
    for c in range(N_CORES):
        out[c // HPC] += np.asarray(res.results[c]["po"], np.float32).T
    out += bo
    return out
